# revision 11
# baseline (speedup 1.0000x reference)
"""Trainium2 Bass kernel for nn_Bilinear (B=256, U=512, D0=512, D1=1024).

out[b,u] = sum_{i,j} x[b,i] * w[u,i,j] * y[b,j] + bias[u]

Strategy (8-way tensor parallel over units U):
  - Shard w along U: 64 units per core. Replicate x, y.
  - Per core, per unit u:
      GEMM1 on TensorE:  XW[u] = X @ W[u]        (256x512 @ 512x1024)
        lhsT = X^T tiles (bf16, stationary, reused across all u)
        rhs  = W[u] tiles (bf16, streamed from HBM in natural (i,j) layout)
        accumulate fp32 in PSUM (two 512-wide n-slices -> one 2-bank tile)
      Contraction on VectorE + ScalarE:
        prod = XW[u] * y      (tensor_tensor mult, PSUM x SBUF -> SBUF)
        out[:, u] = reduce_j prod   (ScalarE activation Copy with accum_out)
  - Host: gather per-core (256, 64) outputs, concat along U, add bias.

W is cast to bf16 on host (halves HBM traffic; fp32 accumulate in PSUM
keeps the j/i contraction exact). y stays fp32 through the second
contraction on DVE (fp32 internal).
"""

import numpy as np
import ml_dtypes

import concourse.mybir as mybir
import concourse.tile as tile
from concourse import bacc
from concourse.bass_utils import run_bass_kernel_spmd

BF16 = mybir.dt.bfloat16
F32 = mybir.dt.float32

B, U, D0, D1 = 256, 512, 512, 1024
NCORES = 8
U_SH = U // NCORES          # 64 units per core
KT = D0 // 128              # 4 k-tiles (contraction i)
MT = B // 128               # 2 m-tiles (batch b)
NT = D1 // 512              # 2 n-slices (free j) per psum tile

_CACHE = {}


def build_program(w_bufs=6):
    nc = bacc.Bacc("TRN2", debug=False)
    w_d = nc.dram_tensor("w", (U_SH, D0, D1), BF16, kind="ExternalInput").ap()
    xT_d = nc.dram_tensor("xT", (D0, B), BF16, kind="ExternalInput").ap()
    y_d = nc.dram_tensor("y", (B, D1), F32, kind="ExternalInput").ap()
    out_d = nc.dram_tensor("out", (B, U_SH), F32, kind="ExternalOutput").ap()

    with tile.TileContext(nc) as tc:
        with (
            tc.tile_pool(name="const", bufs=1) as cpool,
            tc.tile_pool(name="wpool", bufs=w_bufs) as wpool,
            tc.tile_pool(name="ppool", bufs=3, space="PSUM") as ppool,
            tc.tile_pool(name="warmp", bufs=1, space="PSUM") as warmpool,
            tc.tile_pool(name="spool", bufs=4) as spool,
            tc.tile_pool(name="dpool", bufs=2) as dpool,
            tc.tile_pool(name="opool", bufs=1) as opool,
        ):
            # HAM warmup: ~3.5us of dummy matmuls on a memset tile (no DMA
            # dependency). Results go to the first psum-pool slot, which is
            # recycled by the main loop afterwards. Gets the PE clock to
            # 8/8 before the real matmul stream starts, overlapping the
            # initial W DMAs.
            warm_sb = cpool.tile([128, 640], BF16)
            nc.gpsimd.memset(warm_sb[:], 0.0)
            warm_ps = warmpool.tile([128, 512], F32)
            for _ in range(14):
                nc.tensor.matmul(warm_ps[:, 0:512], warm_sb[:, 512:640],
                                 warm_sb[:, 0:512], start=True, stop=True)

            # First W slab on the Scalar HWDGE ring, in parallel with xT
            # on the Sync ring.
            w_tiles = {}
            w_sb = wpool.tile([128, KT * D1], BF16, tag="w_sb")
            for k in range(KT):
                nc.scalar.dma_start(w_sb[:, k * D1:(k + 1) * D1],
                                    w_d[0, k * 128:(k + 1) * 128, :])
            w_tiles[0] = w_sb

            # X^T stationary: (i=512, b=256) -> 4 k-tiles of (128, 256)
            xT_sb = cpool.tile([128, KT * B], BF16)
            for k in range(KT):
                nc.sync.dma_start(xT_sb[:, k * B:(k + 1) * B],
                                  xT_d[k * 128:(k + 1) * 128, :])

            # W prefetch for the next units before the y DMAs.
            for u in (1, 2, 3):
                w_sb = wpool.tile([128, KT * D1], BF16, tag="w_sb")
                for k in range(KT):
                    nc.sync.dma_start(w_sb[:, k * D1:(k + 1) * D1],
                                      w_d[u, k * 128:(k + 1) * 128, :])
                w_tiles[u] = w_sb

            # y: (b=256, j=1024) fp32 -> 2 m-tiles of (128, 1024).
            # Not needed until the first TENSOR_TENSOR (~13us in).
            y_sb = cpool.tile([128, MT * D1], F32)
            for m in range(MT):
                nc.sync.dma_start(y_sb[:, m * D1:(m + 1) * D1],
                                  y_d[m * 128:(m + 1) * 128, :])
            out_sb = opool.tile([128, MT * U_SH], F32)

            for u in range(U_SH):
                if u in w_tiles:
                    w_sb = w_tiles.pop(u)
                else:
                    w_sb = wpool.tile([128, KT * D1], BF16, tag="w_sb")
                    for k in range(KT):
                        nc.sync.dma_start(w_sb[:, k * D1:(k + 1) * D1],
                                          w_d[u, k * 128:(k + 1) * 128, :])
                for m in range(MT):
                    ps = ppool.tile([128, D1], F32, tag="ps")  # 2 PSUM banks
                    for n in range(NT):
                        for k in range(KT):
                            nc.tensor.matmul(
                                ps[:, n * 512:(n + 1) * 512],
                                xT_sb[:, k * B + m * 128: k * B + (m + 1) * 128],
                                w_sb[:, k * D1 + n * 512: k * D1 + (n + 1) * 512],
                                start=(k == 0), stop=(k == KT - 1),
                            )
                    prod = spool.tile([128, D1], F32)
                    nc.vector.tensor_tensor(
                        out=prod[:], in0=ps[:],
                        in1=y_sb[:, m * D1:(m + 1) * D1],
                        op=mybir.AluOpType.mult)
                    dummy = dpool.tile([128, D1], F32)
                    nc.scalar.activation(
                        dummy[:], prod[:], mybir.ActivationFunctionType.Copy,
                        accum_out=out_sb[:, m * U_SH + u: m * U_SH + u + 1])
            for m in range(MT):
                nc.sync.dma_start(out_d[m * 128:(m + 1) * 128, :],
                                  out_sb[:, m * U_SH:(m + 1) * U_SH])
    nc.compile()
    return nc


def _get_program():
    if "nc" not in _CACHE:
        _CACHE["nc"] = build_program()
    return _CACHE["nc"]


def kernel(x, y, w, b):
    x = np.asarray(x, dtype=np.float32)
    y = np.asarray(y, dtype=np.float32)
    w = np.asarray(w)
    b = np.asarray(b, dtype=np.float32)

    nc = _get_program()

    xT = np.ascontiguousarray(x.T).astype(ml_dtypes.bfloat16)
    y32 = np.ascontiguousarray(y)
    in_maps = []
    for c in range(NCORES):
        w_sh = np.asarray(w[c * U_SH:(c + 1) * U_SH]).astype(ml_dtypes.bfloat16)
        in_maps.append({"w": w_sh, "xT": xT, "y": y32})

    res = run_bass_kernel_spmd(nc, in_maps, core_ids=list(range(NCORES)))
    out = np.concatenate([res.results[c]["out"] for c in range(NCORES)], axis=1)
    out = out + b[None, :]
    return out.astype(np.float32)


# revision 12
# speedup vs baseline: 1.0240x; 1.0240x over previous
"""Trainium2 Bass kernel for nn_Bilinear (B=256, U=512, D0=512, D1=1024).

out[b,u] = sum_{i,j} x[b,i] * w[u,i,j] * y[b,j] + bias[u]

Strategy (8-way tensor parallel over units U):
  - Shard w along U: 64 units per core. Replicate x, y.
  - Per core, per unit u:
      GEMM1 on TensorE:  XW[u] = X @ W[u]        (256x512 @ 512x1024)
        lhsT = X^T tiles (bf16, stationary, reused across all u)
        rhs  = W[u] tiles (bf16, streamed from HBM in natural (i,j) layout)
        accumulate fp32 in PSUM (two 512-wide n-slices -> one 2-bank tile)
      Contraction on VectorE + ScalarE:
        prod = XW[u] * y      (tensor_tensor mult, PSUM x SBUF -> SBUF)
        out[:, u] = reduce_j prod   (ScalarE activation Copy with accum_out)
  - Host: gather per-core (256, 64) outputs, concat along U, add bias.

W is cast to bf16 on host (halves HBM traffic; fp32 accumulate in PSUM
keeps the j/i contraction exact). y stays fp32 through the second
contraction on DVE (fp32 internal).
"""

import numpy as np
import ml_dtypes

import concourse.mybir as mybir
import concourse.tile as tile
from concourse import bacc
from concourse.bass_utils import run_bass_kernel_spmd

BF16 = mybir.dt.bfloat16
F32 = mybir.dt.float32

B, U, D0, D1 = 256, 512, 512, 1024
NCORES = 8
U_SH = U // NCORES          # 64 units per core
KT = D0 // 128              # 4 k-tiles (contraction i)
MT = B // 128               # 2 m-tiles (batch b)
NT = D1 // 512              # 2 n-slices (free j) per psum tile

_CACHE = {}


def build_program(w_bufs=6):
    nc = bacc.Bacc("TRN2", debug=False)
    w_d = nc.dram_tensor("w", (U_SH, D0, D1), BF16, kind="ExternalInput").ap()
    xT_d = nc.dram_tensor("xT", (D0, B), BF16, kind="ExternalInput").ap()
    y_d = nc.dram_tensor("y", (B, D1), F32, kind="ExternalInput").ap()
    out_d = nc.dram_tensor("out", (B, U_SH), F32, kind="ExternalOutput").ap()

    with tile.TileContext(nc) as tc:
        with (
            tc.tile_pool(name="const", bufs=1) as cpool,
            tc.tile_pool(name="wpool", bufs=w_bufs) as wpool,
            tc.tile_pool(name="ppool", bufs=3, space="PSUM") as ppool,
            tc.tile_pool(name="warmp", bufs=1, space="PSUM") as warmpool,
            tc.tile_pool(name="spool", bufs=4) as spool,
            tc.tile_pool(name="dpool", bufs=2) as dpool,
            tc.tile_pool(name="opool", bufs=1) as opool,
        ):
            # HAM warmup: ~3.5us of dummy matmuls on a memset tile (no DMA
            # dependency). Results go to the first psum-pool slot, which is
            # recycled by the main loop afterwards. Gets the PE clock to
            # 8/8 before the real matmul stream starts, overlapping the
            # initial W DMAs.
            warm_sb = cpool.tile([128, 640], BF16)
            nc.gpsimd.memset(warm_sb[:], 0.0)
            warm_ps = warmpool.tile([128, 512], F32)
            for _ in range(14):
                nc.tensor.matmul(warm_ps[:, 0:512], warm_sb[:, 512:640],
                                 warm_sb[:, 0:512], start=True, stop=True)

            # First W slab on the Scalar HWDGE ring, in parallel with xT
            # on the Sync ring.
            w_tiles = {}
            w_sb = wpool.tile([128, KT * D1], BF16, tag="w_sb")
            for k in range(KT):
                nc.scalar.dma_start(w_sb[:, k * D1:(k + 1) * D1],
                                    w_d[0, k * 128:(k + 1) * 128, :])
            w_tiles[0] = w_sb

            # X^T stationary: (i=512, b=256) -> 4 k-tiles of (128, 256)
            xT_sb = cpool.tile([128, KT * B], BF16)
            for k in range(KT):
                nc.sync.dma_start(xT_sb[:, k * B:(k + 1) * B],
                                  xT_d[k * 128:(k + 1) * 128, :])

            # y: (b=256, j=1024) fp32 -> 2 m-tiles of (128, 1024).
            # Needed by the first TENSOR_TENSOR (~13us in) — keep it ahead
            # of the bulk W prefetch.
            y_sb = cpool.tile([128, MT * D1], F32)
            for m in range(MT):
                nc.sync.dma_start(y_sb[:, m * D1:(m + 1) * D1],
                                  y_d[m * 128:(m + 1) * 128, :])

            # W prefetch for the next units.
            for u in (1, 2, 3):
                w_sb = wpool.tile([128, KT * D1], BF16, tag="w_sb")
                for k in range(KT):
                    nc.sync.dma_start(w_sb[:, k * D1:(k + 1) * D1],
                                      w_d[u, k * 128:(k + 1) * 128, :])
                w_tiles[u] = w_sb

            out_sb = opool.tile([128, MT * U_SH], F32)

            for u in range(U_SH):
                if u in w_tiles:
                    w_sb = w_tiles.pop(u)
                else:
                    w_sb = wpool.tile([128, KT * D1], BF16, tag="w_sb")
                    for k in range(KT):
                        nc.sync.dma_start(w_sb[:, k * D1:(k + 1) * D1],
                                          w_d[u, k * 128:(k + 1) * 128, :])
                for m in range(MT):
                    ps = ppool.tile([128, D1], F32, tag="ps")  # 2 PSUM banks
                    for n in range(NT):
                        for k in range(KT):
                            nc.tensor.matmul(
                                ps[:, n * 512:(n + 1) * 512],
                                xT_sb[:, k * B + m * 128: k * B + (m + 1) * 128],
                                w_sb[:, k * D1 + n * 512: k * D1 + (n + 1) * 512],
                                start=(k == 0), stop=(k == KT - 1),
                            )
                    prod = spool.tile([128, D1], F32)
                    nc.vector.tensor_tensor(
                        out=prod[:], in0=ps[:],
                        in1=y_sb[:, m * D1:(m + 1) * D1],
                        op=mybir.AluOpType.mult)
                    dummy = dpool.tile([128, D1], F32)
                    nc.scalar.activation(
                        dummy[:], prod[:], mybir.ActivationFunctionType.Copy,
                        accum_out=out_sb[:, m * U_SH + u: m * U_SH + u + 1])
            for m in range(MT):
                nc.sync.dma_start(out_d[m * 128:(m + 1) * 128, :],
                                  out_sb[:, m * U_SH:(m + 1) * U_SH])
    nc.compile()
    return nc


def _get_program():
    if "nc" not in _CACHE:
        _CACHE["nc"] = build_program()
    return _CACHE["nc"]


def kernel(x, y, w, b):
    x = np.asarray(x, dtype=np.float32)
    y = np.asarray(y, dtype=np.float32)
    w = np.asarray(w)
    b = np.asarray(b, dtype=np.float32)

    nc = _get_program()

    xT = np.ascontiguousarray(x.T).astype(ml_dtypes.bfloat16)
    y32 = np.ascontiguousarray(y)
    in_maps = []
    for c in range(NCORES):
        w_sh = np.asarray(w[c * U_SH:(c + 1) * U_SH]).astype(ml_dtypes.bfloat16)
        in_maps.append({"w": w_sh, "xT": xT, "y": y32})

    res = run_bass_kernel_spmd(nc, in_maps, core_ids=list(range(NCORES)))
    out = np.concatenate([res.results[c]["out"] for c in range(NCORES)], axis=1)
    out = out + b[None, :]
    return out.astype(np.float32)


# revision 14
# speedup vs baseline: 1.0251x; 1.0011x over previous
"""Trainium2 Bass kernel for nn_Bilinear (B=256, U=512, D0=512, D1=1024).

out[b,u] = sum_{i,j} x[b,i] * w[u,i,j] * y[b,j] + bias[u]

Strategy (8-way tensor parallel over units U):
  - Shard w along U: 64 units per core. Replicate x, y.
  - Per core, per unit u:
      GEMM1 on TensorE:  XW[u] = X @ W[u]        (256x512 @ 512x1024)
        lhsT = X^T tiles (bf16, stationary, reused across all u)
        rhs  = W[u] tiles (bf16, streamed from HBM in natural (i,j) layout)
        accumulate fp32 in PSUM (two 512-wide n-slices -> one 2-bank tile)
      Contraction on VectorE + ScalarE:
        prod = XW[u] * y      (tensor_tensor mult, PSUM x SBUF -> SBUF)
        out[:, u] = reduce_j prod   (ScalarE activation Copy with accum_out)
  - Host: gather per-core (256, 64) outputs, concat along U, add bias.

W is cast to bf16 on host (halves HBM traffic; fp32 accumulate in PSUM
keeps the j/i contraction exact). y stays fp32 through the second
contraction on DVE (fp32 internal).
"""

import numpy as np
import ml_dtypes

import concourse.mybir as mybir
import concourse.tile as tile
from concourse import bacc
from concourse.bass_utils import run_bass_kernel_spmd

BF16 = mybir.dt.bfloat16
F32 = mybir.dt.float32

B, U, D0, D1 = 256, 512, 512, 1024
NCORES = 8
U_SH = U // NCORES          # 64 units per core
KT = D0 // 128              # 4 k-tiles (contraction i)
MT = B // 128               # 2 m-tiles (batch b)
NT = D1 // 512              # 2 n-slices (free j) per psum tile

_CACHE = {}


def build_program(w_bufs=6):
    nc = bacc.Bacc("TRN2", debug=False)
    w_d = nc.dram_tensor("w", (U_SH, D0, D1), BF16, kind="ExternalInput").ap()
    xT_d = nc.dram_tensor("xT", (D0, B), BF16, kind="ExternalInput").ap()
    y_d = nc.dram_tensor("y", (B, D1), F32, kind="ExternalInput").ap()
    out_d = nc.dram_tensor("out", (B, U_SH), F32, kind="ExternalOutput").ap()

    with tile.TileContext(nc) as tc:
        with (
            tc.tile_pool(name="const", bufs=1) as cpool,
            tc.tile_pool(name="wpool", bufs=w_bufs) as wpool,
            tc.tile_pool(name="ppool", bufs=3, space="PSUM") as ppool,
            tc.tile_pool(name="warmp", bufs=1, space="PSUM") as warmpool,
            tc.tile_pool(name="spool", bufs=4) as spool,
            tc.tile_pool(name="dpool", bufs=2) as dpool,
            tc.tile_pool(name="opool", bufs=1) as opool,
        ):
            # HAM warmup: ~3.5us of dummy matmuls on a memset tile (no DMA
            # dependency). Results go to the first psum-pool slot, which is
            # recycled by the main loop afterwards. Gets the PE clock to
            # 8/8 before the real matmul stream starts, overlapping the
            # initial W DMAs.
            warm_sb = cpool.tile([128, 640], BF16)
            nc.gpsimd.memset(warm_sb[:], 0.0)
            warm_ps = warmpool.tile([128, 512], F32)
            for _ in range(14):
                nc.tensor.matmul(warm_ps[:, 0:512], warm_sb[:, 512:640],
                                 warm_sb[:, 0:512], start=True, stop=True)

            # First two W slabs on the Scalar HWDGE ring, in parallel with
            # xT/y on the Sync ring.
            w_tiles = {}
            for u in (0, 1):
                w_sb = wpool.tile([128, KT * D1], BF16, tag="w_sb")
                for k in range(KT):
                    nc.scalar.dma_start(w_sb[:, k * D1:(k + 1) * D1],
                                        w_d[u, k * 128:(k + 1) * 128, :])
                w_tiles[u] = w_sb

            # X^T stationary: (i=512, b=256) -> 4 k-tiles of (128, 256)
            xT_sb = cpool.tile([128, KT * B], BF16)
            for k in range(KT):
                nc.sync.dma_start(xT_sb[:, k * B:(k + 1) * B],
                                  xT_d[k * 128:(k + 1) * 128, :])

            # y: (b=256, j=1024) fp32 -> 2 m-tiles of (128, 1024).
            # Needed by the first TENSOR_TENSOR (~13us in) — keep it ahead
            # of the bulk W prefetch.
            y_sb = cpool.tile([128, MT * D1], F32)
            for m in range(MT):
                nc.sync.dma_start(y_sb[:, m * D1:(m + 1) * D1],
                                  y_d[m * 128:(m + 1) * 128, :])

            # W prefetch for the next units.
            for u in (2, 3):
                w_sb = wpool.tile([128, KT * D1], BF16, tag="w_sb")
                for k in range(KT):
                    nc.sync.dma_start(w_sb[:, k * D1:(k + 1) * D1],
                                      w_d[u, k * 128:(k + 1) * 128, :])
                w_tiles[u] = w_sb

            out_sb = opool.tile([128, MT * U_SH], F32)

            for u in range(U_SH):
                if u in w_tiles:
                    w_sb = w_tiles.pop(u)
                else:
                    w_sb = wpool.tile([128, KT * D1], BF16, tag="w_sb")
                    for k in range(KT):
                        nc.sync.dma_start(w_sb[:, k * D1:(k + 1) * D1],
                                          w_d[u, k * 128:(k + 1) * 128, :])
                for m in range(MT):
                    ps = ppool.tile([128, D1], F32, tag="ps")  # 2 PSUM banks
                    for n in range(NT):
                        for k in range(KT):
                            nc.tensor.matmul(
                                ps[:, n * 512:(n + 1) * 512],
                                xT_sb[:, k * B + m * 128: k * B + (m + 1) * 128],
                                w_sb[:, k * D1 + n * 512: k * D1 + (n + 1) * 512],
                                start=(k == 0), stop=(k == KT - 1),
                            )
                    prod = spool.tile([128, D1], F32)
                    nc.vector.tensor_tensor(
                        out=prod[:], in0=ps[:],
                        in1=y_sb[:, m * D1:(m + 1) * D1],
                        op=mybir.AluOpType.mult)
                    dummy = dpool.tile([128, D1], F32)
                    nc.scalar.activation(
                        dummy[:], prod[:], mybir.ActivationFunctionType.Copy,
                        accum_out=out_sb[:, m * U_SH + u: m * U_SH + u + 1])
            for m in range(MT):
                nc.sync.dma_start(out_d[m * 128:(m + 1) * 128, :],
                                  out_sb[:, m * U_SH:(m + 1) * U_SH])
    nc.compile()
    return nc


def _get_program():
    if "nc" not in _CACHE:
        _CACHE["nc"] = build_program()
    return _CACHE["nc"]


def kernel(x, y, w, b):
    x = np.asarray(x, dtype=np.float32)
    y = np.asarray(y, dtype=np.float32)
    w = np.asarray(w)
    b = np.asarray(b, dtype=np.float32)

    nc = _get_program()

    xT = np.ascontiguousarray(x.T).astype(ml_dtypes.bfloat16)
    y32 = np.ascontiguousarray(y)
    in_maps = []
    for c in range(NCORES):
        w_sh = np.asarray(w[c * U_SH:(c + 1) * U_SH]).astype(ml_dtypes.bfloat16)
        in_maps.append({"w": w_sh, "xT": xT, "y": y32})

    res = run_bass_kernel_spmd(nc, in_maps, core_ids=list(range(NCORES)))
    out = np.concatenate([res.results[c]["out"] for c in range(NCORES)], axis=1)
    out = out + b[None, :]
    return out.astype(np.float32)
